# revision 11
# baseline (speedup 1.0000x reference)
"""Trainium2 Bass kernel for FIRResample2d (upfirdn2d, up=2, down=1, pad=(2,1),
4x4 FIR kernel).

Full input x: (16, 128, 128, 128) f32 NCHW -> output (16, 128, 256, 256) f32.

Strategy
--------
Data-parallel over 8 NeuronCores: core i processes batches [2i, 2i+1]
(no cross-device communication; the op is per-(batch, channel) spatial
filtering).

Math: with up=2, pad=(2,1) and a 4-tap kernel the op is polyphase:
    out[2m]   = k[3]*x[m-1] + k[1]*x[m]
    out[2m+1] = k[2]*x[m]   + k[0]*x[m+1]
per axis (with zero boundary).  The 4x4 kernel produced by
make_kernel([1,3,3,1], gain) is rank-1 (outer(ky, kx)), so the 2-D op
separates into a horizontal 2-tap pass followed by a vertical 2-tap pass.
We SVD the given fir_kernel at runtime into rank-1 components (always
exactly 1 for this problem) and run the separable device kernel per
component.

Performance design (the kernel is DMA-bound, ~40 MiB/core at ~360 GB/s):
  * All device I/O is fp16: the host casts x f32->f16 and the result
    f16->f32.  This halves HBM traffic; the fp16 rounding error
    (~2^-11 per element) is far inside the 2e-2 relative-error budget.
  * Whole-image fp16 ring tiles in SBUF per batch: x plane [130,128]
    (zero halo rows), prescaled plane xs [130,130] (zero halo rows AND
    columns, so the horizontal pass needs no boundary special cases),
    t plane [130,256].  Strips of 16 rows pipeline DMA-in / prescale /
    h-pass / v-pass / DMA-out through them; sub-tile dependency tracking
    keeps disjoint row ranges independent.
  * The element-wise work (one scalar_tensor_tensor per produced element)
    is split across BOTH vector engines: DVE (1.04 ns/elem/partition) and
    Pool/GpSimd (0.83/0.6 = 1.39 ns/elem/partition), with column-granular
    fractions chosen so both engines finish a strip in the same time.
    The ACT engine does the single pre-scaled copy.
  * Input DMAs ride the SP HWDGE ring, output DMAs the ACT ring, each
    output strip split by row parity (512B-contiguous runs either way,
    so all DMAs run at full descriptor rate).
"""

import numpy as np

B_FULL, C, H, W = 16, 128, 128, 128
OH, OW = 2 * H, 2 * W
N_CORES = 8
B_PER_CORE = B_FULL // N_CORES
HS = 16  # strip height (input rows per strip)

# fraction of each op family's columns computed on DVE (rest on Pool/GpSimd).
# NOTE: TensorScalarPtr is NOT legal on the Pool engine on real TRN2 (walrus
# ISA check rejects it), so these must all stay 1.0 = all-DVE.
FRAC = {"he": 1.0, "ho": 1.0, "ve": 1.0, "vo": 1.0}

_PROG_CACHE: dict = {}


def _split_multi_waits(nc):
    """The walrus build here supports a single sync-wait per instruction;
    hoist extra waits onto preceding same-engine NOPs (the canonical raw-bass
    idiom: standalone waits ahead of the gated instruction)."""
    import concourse.mybir as mybir

    for f in nc.m.functions:
        for bb in f.blocks:
            new_insts = []
            for inst in bb.instructions:
                si = inst.sync_info
                waits = list(si.on_wait) if si is not None else []
                if len(waits) > 1:
                    for i, w in enumerate(waits[:-1]):
                        nop = mybir.InstNoOp(
                            name=f"{inst.name}-sw{i}",
                            engine=inst.engine,
                            sync_info=mybir.SyncInfo(on_wait=[w], on_update=[]),
                        )
                        nc.register_instruction(nop, overwrite=True)
                        new_insts.append(nop)
                    si.on_wait = [waits[-1]]
                new_insts.append(inst)
            bb.instructions = new_insts


def _build_fir_program(
    ky, kx, b_per_core, c, h, w, hs, reps=1, loop_n=1, mode="full", frac=None
):
    """Build the per-core Bass program for one separable component.

    ky, kx: length-4 1-D tap vectors (floats), kernel2d = outer(ky, kx).
    Input "x" [b_per_core, c, h, w] f16, output "out" [b_per_core, c, 2h, 2w]
    f16 (the host does the f32<->f16 casts).
    """
    import concourse.bass as bass
    import concourse.mybir as mybir
    from concourse.tile import TileContext

    f16 = mybir.dt.float16
    mult = mybir.AluOpType.mult
    add = mybir.AluOpType.add
    if frac is None:
        frac = FRAC

    kx0, kx1, kx2, kx3 = (float(v) for v in kx)
    ky0, ky1, ky2, ky3 = (float(v) for v in ky)
    # symmetric-separable fast path: fold ky1*kx1 into the single prescaled
    # plane so neither pass needs extra pre-scaled copies.
    sym = kx1 == kx2 and ky1 == ky2 and ky1 != 0.0 and kx1 != 0.0

    oh, ow = 2 * h, 2 * w
    assert h % hs == 0
    # strip heights: fine-grained at the start (fast pipeline fill) and at
    # the end (short drain tail), full hs in the middle
    heights = [hs // 4, hs // 4, hs // 2]
    mid = (h - 2 * hs) // hs
    heights += [hs] * mid + [hs // 2, hs // 4, hs // 4]
    assert sum(heights) == h, heights

    nc = bass.Bass()
    x = nc.dram_tensor("x", [b_per_core, c, h, w], f16, kind="ExternalInput")
    out = nc.dram_tensor("out", [b_per_core, c, oh, ow], f16, kind="ExternalOutput")

    import contextlib

    # mode: "full" | "dma" (DMAs only — wire-speed calibration)
    #              | "compute" (engines only — DVE/ACT-speed calibration)
    emit_dma = mode != "compute"
    emit_compute = mode != "dma"

    def stt_split(f_dve, ncols_op, mk, scalar):
        """Emit one stt column-split across DVE ([0, cs)) and Pool ([cs, n))."""
        cs = int(round(f_dve * ncols_op))
        for eng, c0, c1 in ((nc.vector, 0, cs), (nc.gpsimd, cs, ncols_op)):
            if c1 > c0:
                o, i0, i1 = mk(c0, c1)
                eng.scalar_tensor_tensor(
                    out=o, in0=i0, scalar=scalar, in1=i1, op0=mult, op1=add
                )

    with TileContext(nc) as tc:
        with tc.tile_pool(name="pool", bufs=1) as pool, (
            tc.For_i(0, loop_n, 1) if loop_n > 1 else contextlib.nullcontext()
        ):
            # whole-image ring tiles shared by both batches; slot r holds
            # image row r-1 (rows -1 and h are the zero halo).  Hoisted out
            # of the batch loop so batch b+1's strips overlap batch b's tail
            # (sub-tile deps keep disjoint row ranges independent).
            xf = pool.tile([c, h + 2, w], f16, name="xf")
            xs = pool.tile([c, h + 2, w + 2], f16, name="xs")
            tf = pool.tile([c, h + 2, ow], f16, name="tf")
            if not sym and emit_compute:
                xp = pool.tile([c, h + 2, w + 2], f16, name="xp")
                xs2 = pool.tile([c, h + 2, w + 2], f16, name="xs2")
                ta = pool.tile([c, h + 2, ow], f16, name="ta")
                tb = pool.tile([c, h + 2, ow], f16, name="tb")
            if emit_compute:
                # zero halos: x rows, t rows, xs columns (once; strips only
                # ever rewrite interior rows/columns)
                nc.gpsimd.memset(xf[:, 0:1, :], 0.0)
                nc.gpsimd.memset(xf[:, h + 1 : h + 2, :], 0.0)
                nc.gpsimd.memset(tf[:, 0:1, :], 0.0)
                nc.gpsimd.memset(tf[:, h + 1 : h + 2, :], 0.0)
                nc.gpsimd.memset(xs[:, :, 0:1], 0.0)
                nc.gpsimd.memset(xs[:, :, w + 1 : w + 2], 0.0)
                if not sym:
                    for tile_ in (xp, xs2):
                        nc.gpsimd.memset(tile_[:, :, 0:1], 0.0)
                        nc.gpsimd.memset(tile_[:, :, w + 1 : w + 2], 0.0)
                    for tile_ in (ta, tb):
                        nc.gpsimd.memset(tile_[:, 0:1, :], 0.0)
                        nc.gpsimd.memset(tile_[:, h + 1 : h + 2, :], 0.0)
            # flat work list: (b, si, sh, m0, a, bnd) per strip
            items = []
            for _rep in range(reps):
                for b in range(b_per_core):
                    m0 = 0
                    for si, sh in enumerate(heights):
                        a = 1 if si == 0 else m0 + 2
                        bnd = min(m0 + sh + 2, h + 1)
                        items.append((b, si, sh, m0, a, bnd))
                        m0 += sh

            def emit_load(item):
                b, si, sh, m0, a, bnd = item
                if emit_dma:
                    nc.sync.dma_start(
                        out=xf[:, a:bnd, :], in_=x[b, :, a - 1 : bnd - 1, :]
                    )
                if not emit_dma and si == 0:
                    nc.gpsimd.memset(xf[:, 0:1, 0:8], 0.0)
                if not emit_compute:
                    return
                # --- prescale (ACT): xs = (ky1*kx1)*x, zero-padded
                s_xs = ky1 * kx1 if sym else kx1
                nc.scalar.mul(xs[:, a:bnd, 1 : w + 1], xf[:, a:bnd, :], s_xs)
                if not sym:
                    nc.scalar.copy(xp[:, a:bnd, 1 : w + 1], xf[:, a:bnd, :])
                    nc.scalar.mul(xs2[:, a:bnd, 1 : w + 1], xf[:, a:bnd, :], kx2)

            def emit_compute_store(item):
                b, si, sh, m0, a, bnd = item
                # one ring of full-height slots; short strips use the
                # leading rows of a slot
                obuf = pool.tile([c, 2 * hs, ow], f16, name="obuf", bufs=3)
                if not emit_compute:
                    # calibration mode: tiles need >=1 writer for a slot
                    nc.gpsimd.memset(obuf[:, 0:1, 0:8], 0.0)
                else:
                    if sym:
                        pA = pB = xs
                        sh_e, sh_o = kx3 / kx1, kx0 / kx1
                    else:
                        pA = xp
                        pB = xs2
                        sh_e, sh_o = kx3, kx0
                    # --- horizontal pass into t slots [a, bnd)
                    # t[r, 2n]   = sh_e * pA[r, n-1] + xs[r, n]
                    # t[r, 2n+1] = sh_o * pA[r, n+1] + pB[r, n]
                    stt_split(
                        frac["he"],
                        w,
                        lambda c0, c1: (
                            tf[:, a:bnd, 2 * c0 : 2 * c1 : 2],
                            pA[:, a:bnd, c0:c1],
                            xs[:, a:bnd, c0 + 1 : c1 + 1],
                        ),
                        sh_e,
                    )
                    stt_split(
                        frac["ho"],
                        w,
                        lambda c0, c1: (
                            tf[:, a:bnd, 2 * c0 + 1 : 2 * c1 : 2],
                            pA[:, a:bnd, c0 + 2 : c1 + 2],
                            pB[:, a:bnd, c0 + 1 : c1 + 1],
                        ),
                        sh_o,
                    )
                    # --- vertical pass: out rows [2*m0, 2*m0+2*sh)
                    # out[2m]   = sv_e * tA[m]   + tB[m+1]   (slots)
                    # out[2m+1] = sv_o * tA[m+2] + tB[m+1]
                    if sym:
                        sv_e, sv_o = ky3 / ky1, ky0 / ky1
                    else:
                        nc.scalar.mul(ta[:, a:bnd, :], tf[:, a:bnd, :], ky1)
                        nc.scalar.mul(tb[:, a:bnd, :], tf[:, a:bnd, :], ky2)
                        sv_e, sv_o = ky3, ky0
                    tA = tf
                    tBe = tf if sym else ta
                    tBo = tf if sym else tb
                    stt_split(
                        frac["ve"],
                        ow,
                        lambda c0, c1: (
                            obuf[:, 0 : 2 * sh : 2, c0:c1],
                            tA[:, m0 : m0 + sh, c0:c1],
                            tBe[:, m0 + 1 : m0 + sh + 1, c0:c1],
                        ),
                        sv_e,
                    )
                    stt_split(
                        frac["vo"],
                        ow,
                        lambda c0, c1: (
                            obuf[:, 1 : 2 * sh : 2, c0:c1],
                            tA[:, m0 + 2 : m0 + sh + 2, c0:c1],
                            tBo[:, m0 + 1 : m0 + sh + 1, c0:c1],
                        ),
                        sv_o,
                    )
                # output DMAs on the second HWDGE ring (ACT), split by
                # row parity so the even-row store overlaps the odd-row
                # compute and the drain tail halves
                if emit_dma:
                    nc.scalar.dma_start(
                        out=out[b, :, 2 * m0 : 2 * m0 + 2 * sh : 2, :],
                        in_=obuf[:, 0 : 2 * sh : 2, :],
                    )
                    nc.scalar.dma_start(
                        out=out[b, :, 2 * m0 + 1 : 2 * m0 + 2 * sh : 2, :],
                        in_=obuf[:, 1 : 2 * sh : 2, :],
                    )

            # software pipelining: loads/prescales run LOOKAHEAD strips ahead
            # of compute+store in every queue's dispatch order, so a prescale
            # never sits behind an output-DMA dispatch that is still waiting
            # on its strip's v-pass (batch-boundary ladder stall)
            LOOKAHEAD = 3
            for k in range(len(items) + LOOKAHEAD):
                if k < len(items):
                    emit_load(items[k])
                if k >= LOOKAHEAD:
                    emit_compute_store(items[k - LOOKAHEAD])
    _split_multi_waits(nc)
    return nc


def _separable_components(k2: np.ndarray):
    """Decompose a 4x4 kernel into rank-1 (ky, kx) components via SVD.

    For this problem's kernel (outer product of [1,3,3,1] taps) there is
    exactly one component; the general path is correctness insurance.
    """
    k64 = np.asarray(k2, dtype=np.float64)
    u, s, vt = np.linalg.svd(k64)
    comps = []
    if s[0] == 0.0:
        return comps
    for i in range(len(s)):
        if s[i] <= 1e-12 * s[0]:
            break
        ky = u[:, i] * np.sqrt(s[i])
        kx = vt[i] * np.sqrt(s[i])
        # sign convention: make the largest-|.| entry of ky positive
        if ky[np.argmax(np.abs(ky))] < 0:
            ky, kx = -ky, -kx
        # snap numerically-symmetric taps so the builder's fast path fires
        for v in (ky, kx):
            if abs(v[1] - v[2]) <= 1e-6 * (abs(v[1]) + abs(v[2])):
                v[1] = v[2] = (v[1] + v[2]) / 2
            if abs(v[0] - v[3]) <= 1e-6 * (abs(v[0]) + abs(v[3]) + 1e-300):
                v[0] = v[3] = (v[0] + v[3]) / 2
        comps.append((ky, kx))
    return comps


def _get_program(ky, kx, reps=1):
    key = (tuple(np.float32(v) for v in ky), tuple(np.float32(v) for v in kx), reps)
    prog = _PROG_CACHE.get(key)
    if prog is None:
        prog = _build_fir_program(ky, kx, B_PER_CORE, C, H, W, HS, reps=reps)
        _PROG_CACHE[key] = prog
    return prog


def _run_spmd(nc, x: np.ndarray) -> np.ndarray:
    from concourse.bass_utils import run_bass_kernel_spmd

    x16 = np.ascontiguousarray(x, dtype=np.float16)
    in_maps = [
        {"x": np.ascontiguousarray(x16[i * B_PER_CORE : (i + 1) * B_PER_CORE])}
        for i in range(N_CORES)
    ]
    res = run_bass_kernel_spmd(nc, in_maps, core_ids=list(range(N_CORES)))
    return np.concatenate([r["out"] for r in res.results], axis=0)


def kernel(x: np.ndarray, fir_kernel: np.ndarray) -> np.ndarray:
    x = np.asarray(x, dtype=np.float32)
    k2 = np.asarray(fir_kernel, dtype=np.float32)
    assert x.shape == (B_FULL, C, H, W), x.shape
    assert k2.shape == (4, 4), k2.shape

    comps = _separable_components(k2)
    if not comps:
        return np.zeros((B_FULL, C, OH, OW), dtype=np.float32)

    acc = None
    for ky, kx in comps:
        y = _run_spmd(_get_program(ky, kx), x).astype(np.float32)
        acc = y if acc is None else acc + y
    return acc.astype(np.float32, copy=False)
